# revision 17
# baseline (speedup 1.0000x reference)
"""Trainium2 Bass kernel for nn_CrossAttentionLayer (m=n=1024, d=2048).

Math:  f = relu(term1 + term23 + term4 + ffn_b), where with
W1..W4 = ffn_w.reshape(n, 4, d) per-candidate blocks:
  term1  = sum_i u_p[i] . W1[i]                      (host: tiny scalar dot)
  term23 = <softmax_rows(S),    (W2 + u_p*W3) @ u_c.T>_F     (S = [m,n] logits)
  term4  = <softmax_rows(S.T),  (u_c*W4)      @ u_p.T>_F
Row-constant offsets cancel inside row-softmax, and the remaining column
offset folds into the matmul:  softmax_k(S[i,:]) = softmax_k((u_p[i]*w3 + w2) @ u_c.T).

Both inner products have the identical SPMD shape
  result = sum_i [ sum_k exp(T[i,k]) * M2[i,k] ] / [ sum_k exp(T[i,k]) ]
  T  = ASp @ B.T,   M2 = C @ B.T
with ASp = A*w3 + a2 and C = C1 + A*C2 folded on the host, so 8 cores run
ONE program on different operands:
  cores 0-3 (mention shard I of 256): A=u_p[I], B=u_c, C=(W2+u_p*W3)[I], a2=w2
  cores 4-7 (candidate shard J):      A=u_c[J], B=u_p, C=(u_c*W4)[J],    a2=w1
Operands are pre-transposed ([d, rows]), quantized to fp8e4m3 with static
scales (SA for ASp, SC for C; exp() descales T via the activation's scale
input, the host descales gz by SC), and packed p-major [128, chunks*cols]
so every DMA is one contiguous multi-KB run per partition.  Matmuls run in
fp8 DoubleRow perf mode (2 contraction rows per partition per cycle).
Per-core outputs are tiny gz row-vectors; the host reduces them, adds
term1 + bias, and applies relu.
"""

import sys

sys.path.insert(0, "/opt/trn_rl_repo")

import ml_dtypes
import numpy as np

import concourse.bass as bass
import concourse.tile as tile
from concourse import mybir
from concourse.bass_utils import run_bass_kernel_spmd

F32 = mybir.dt.float32
F8 = mybir.dt.float8e4
NP_F8 = ml_dtypes.float8_e4m3

M = 1024  # mentions
N = 1024  # candidates
D = 2048  # feature dim (contraction)
NCORES = 8
ISH = 256  # per-core shard rows (A rows)
CH = D // 128  # 16 contraction chunks of 128
ITILES = ISH // 128  # 2
KH = 512  # rhs free-dim per matmul (PSUM bank width in fp32)
NKH = N // KH  # 2

SA = 32.0  # ASp fp8 scale (entries ~0.03 sigma -> ~1)
SC = 512.0  # C fp8 scale (entries ~0.0015 sigma -> ~0.8)

# ---------------------------------------------------------------------------
# Workaround: the pinned neuronxcc walrus accepts fewer sync waits per
# instruction than Tile's semaphore assignment attaches.  After scheduling,
# hoist excess waits of any over-capacity instruction onto same-engine
# EventSemaphores inserted right before it; each engine executes its stream
# in order, so the waits still gate the instruction.
_DEFAULT_CAP = 1
_WAIT_CAPS = {
    "InstTensorScalarPtr": 1,
    "InstTensorScalar": 1,
    "InstScalarTensorTensor": 1,
    "InstTensorReduce": 1,
}
_wfix_counter = [0]


def _legalize_waits(nc: bass.Bass) -> None:
    for f in nc.m.functions:
        for bb in f.blocks:
            il = bb.instructions
            out = []
            for inst in il:
                si = inst.sync_info
                waits = list(si.on_wait) if si and si.on_wait else []
                cap = _WAIT_CAPS.get(type(inst).__name__, _DEFAULT_CAP)
                if len(waits) > cap:
                    keep = waits[:cap]
                    for w in waits[cap:]:
                        _wfix_counter[0] += 1
                        out.append(
                            mybir.InstEventSemaphore(
                                name=f"I-wfix-{_wfix_counter[0]}",
                                engine=inst.engine,
                                ins=[],
                                outs=[],
                                sync_info=mybir.SyncInfo(on_wait=[w], on_update=[]),
                            )
                        )
                    inst.sync_info = mybir.SyncInfo(
                        on_wait=keep, on_update=list(si.on_update or [])
                    )
                out.append(inst)
            bb.instructions = out


# ---------------------------------------------------------------------------
def _emit(nc: bass.Bass, tc: tile.TileContext, io: dict) -> None:
    mult = mybir.AluOpType.mult
    add = mybir.AluOpType.add
    DR = mybir.MatmulPerfMode.DoubleRow

    a_r = io["a8"].ap().rearrange("p (c i) -> p c i", c=CH)
    c_r = io["c8"].ap().rearrange("p (c i) -> p c i", c=CH)
    b_r = io["b8"].ap().rearrange("p (h c k) -> p h c k", h=NKH, c=CH)
    zg_r = io["out_zg"].ap().rearrange("p (c z) -> p c z", z=3)

    import contextlib

    ctx = contextlib.ExitStack()
    singles = ctx.enter_context(tc.tile_pool(name="singles", bufs=1))
    scratch = ctx.enter_context(tc.tile_pool(name="scratch", bufs=3))
    psum = ctx.enter_context(tc.tile_pool(name="psum", bufs=2, space="PSUM"))

    a_sb = singles.tile([128, CH, ISH], F8)
    c_sb = singles.tile([128, CH, ISH], F8)
    b_sb = singles.tile([128, NKH, CH, KH], F8)
    # zg[:, col, 0] = Z partials, zg[:, col, 1:3] = G half partials (DVE and
    # GpSimd each reduce half the columns); host divides.
    zg = singles.tile([128, ITILES * NKH, 3], F32)

    # Tensor-engine warmup: the PE array boots in a half-speed p-state and
    # reaches full clock only after ~5 us of GAPLESS execution (idle gaps
    # reset the ramp).  Burn the DMA-wait window on one junk accumulation
    # chain -- accumulating matmuls pipeline back-to-back, so the ramp
    # carries straight into the real stream that follows on the engine.
    wa_sb = singles.tile([128, 2, 128], F8)
    wb_sb = singles.tile([128, 2, 256], F8)
    nc.vector.memset(wa_sb, 0.0)
    nc.vector.memset(wb_sb, 0.0)
    NWARM = 22
    wps = psum.tile([128, 256], F32, tag="tps0")
    for w in range(NWARM):
        nc.tensor.matmul(
            wps, lhsT=wa_sb, rhs=wb_sb, start=(w == 0), stop=(w == NWARM - 1),
            perf_mode=DR,
        )

    # Input DMAs: one queue per source tensor, pieces in consumption order
    # so supply stays ahead of the matmul stream.  Completion notify lags
    # data by ~2.5 us, so the pieces are small (4 chunks = 0.125 MB).
    QC = 4
    for q in range(CH // QC):
        sl = slice(q * QC, (q + 1) * QC)
        nc.sync.dma_start(out=a_sb[:, sl, :], in_=a_r[:, sl, :])
        nc.scalar.dma_start(out=c_sb[:, sl, :], in_=c_r[:, sl, :])
        nc.gpsimd.dma_start(out=b_sb[:, 0, sl, :], in_=b_r[:, 0, sl, :])
    hc = CH // 2
    nc.sync.dma_start(out=b_sb[:, 1, :hc, :], in_=b_r[:, 1, :hc, :])
    nc.scalar.dma_start(out=b_sb[:, 1, hc:, :], in_=b_r[:, 1, hc:, :])

    # Main contraction: T and M2 accumulate over 8 DoubleRow chunks in PSUM,
    # then exp+rowsum (ACT, descaling by 1/SA) and mul+rowsum (DVE) fold k
    # away.  Each group's Z/G partial columns stream out as they finish.
    # Both it-tiles sweep each chunk together (4 matmuls per chunk pair) so
    # per-byte demand stays under the ~420 GB/s DMA supply -- a stall in the
    # tensor stream would also reset the p-state ramp.
    for kh in range(NKH):
        tps0 = psum.tile([128, KH], F32, tag="tps0")
        tps1 = psum.tile([128, KH], F32, tag="tps1")
        mps0 = psum.tile([128, KH], F32, tag="mps0")
        mps1 = psum.tile([128, KH], F32, tag="mps1")
        tps = [tps0, tps1]
        mps = [mps0, mps1]
        for c2 in range(CH // 2):
            sl2 = slice(2 * c2, 2 * c2 + 2)
            rhs = b_sb[:, kh, sl2, :]
            for it in range(ITILES):
                isl = slice(it * 128, (it + 1) * 128)
                nc.tensor.matmul(
                    tps[it],
                    lhsT=a_sb[:, sl2, isl],
                    rhs=rhs,
                    start=(c2 == 0),
                    stop=(c2 == CH // 2 - 1),
                    perf_mode=DR,
                )
                nc.tensor.matmul(
                    mps[it],
                    lhsT=c_sb[:, sl2, isl],
                    rhs=rhs,
                    start=(c2 == 0),
                    stop=(c2 == CH // 2 - 1),
                    perf_mode=DR,
                )
        for it in range(ITILES):
            col = kh * ITILES + it
            ep = scratch.tile([128, KH], F32, tag="ep")
            nc.scalar.activation(
                out=ep,
                in_=tps[it],
                func=mybir.ActivationFunctionType.Exp,
                scale=1.0 / SA,
                accum_out=zg[:, col, 0:1],
            )
            h2 = scratch.tile([128, KH], F32, tag="h")
            nc.vector.scalar_tensor_tensor(
                out=h2,
                in0=ep,
                scalar=1.0,
                in1=mps[it],
                op0=mult,
                op1=mult,
                accum_out=zg[:, col, 1:2],
            )
            nc.sync.dma_start(out=zg_r[:, col, :], in_=zg[:, col, :])
    ctx.close()


def _build() -> bass.Bass:
    nc = bass.Bass()
    io = {}
    io["a8"] = nc.declare_dram_parameter("a8", [128, CH * ISH], F8, isOutput=False)
    io["c8"] = nc.declare_dram_parameter("c8", [128, CH * ISH], F8, isOutput=False)
    io["b8"] = nc.declare_dram_parameter("b8", [128, NKH * CH * KH], F8, isOutput=False)
    io["out_zg"] = nc.declare_dram_parameter(
        "out_zg", [128, 3 * ITILES * NKH], F32, isOutput=True
    )
    with tile.TileContext(nc) as tc:
        _emit(nc, tc, io)
    _legalize_waits(nc)
    return nc


_NC_CACHE: bass.Bass | None = None


def _get_nc() -> bass.Bass:
    global _NC_CACHE
    if _NC_CACHE is None:
        _NC_CACHE = _build()
    return _NC_CACHE


def _q8(a2d: np.ndarray, scale: float) -> np.ndarray:
    return np.clip(a2d * scale, -240.0, 240.0).astype(NP_F8)


def _pack(a2d: np.ndarray) -> np.ndarray:
    """[D, x] (d-major) -> [128, CH*x] p-major (chunk-major per partition)."""
    x = a2d.shape[1]
    return np.ascontiguousarray(
        a2d.reshape(CH, 128, x).transpose(1, 0, 2).reshape(128, CH * x)
    )


def _pack_b(b2d: np.ndarray) -> np.ndarray:
    """[D, N] -> [128, NKH*CH*KH] with per-partition layout [kh][chunk][col]."""
    return np.ascontiguousarray(
        b2d.reshape(CH, 128, NKH, KH).transpose(1, 2, 0, 3).reshape(128, NKH * CH * KH)
    )


def _in_maps(u_p, u_c, w_a, ffn_w):
    u_pT = np.ascontiguousarray(u_p.T)
    u_cT = np.ascontiguousarray(u_c.T)
    W = ffn_w.reshape(N, 4, D)
    wa = w_a[0]
    w1, w2, w3 = wa[:D], wa[D : 2 * D], wa[2 * D :]

    # host-folded operands, [d, rows]
    asp_a = u_pT * w3[:, None] + w2[:, None]
    asp_b = u_cT * w3[:, None] + w1[:, None]
    c_a = W[:, 1, :].T + u_pT * W[:, 2, :].T  # W2 + u_p*W3
    c_b = u_cT * W[:, 3, :].T  # u_c*W4

    b8_a = _pack_b(_q8(u_cT, 1.0))
    b8_b = _pack_b(_q8(u_pT, 1.0))

    maps = []
    for grp, (asp, cc, b8) in enumerate(((asp_a, c_a, b8_a), (asp_b, c_b, b8_b))):
        for ci in range(4):
            sl = slice(ISH * ci, ISH * (ci + 1))
            maps.append(
                {
                    "a8": _pack(_q8(asp[:, sl], SA)),
                    "c8": _pack(_q8(cc[:, sl], SC)),
                    "b8": b8,
                }
            )
    return maps


def kernel(u_p, u_c, w_a, ffn_w, ffn_b, **run_kwargs):
    nc = _get_nc()
    u_p = np.asarray(u_p, np.float32)
    u_c = np.asarray(u_c, np.float32)
    w_a = np.asarray(w_a, np.float32)
    ffn_w = np.asarray(ffn_w, np.float32)
    maps = _in_maps(u_p, u_c, w_a, ffn_w)
    res = run_bass_kernel_spmd(nc, maps, core_ids=list(range(NCORES)), **run_kwargs)
    total = 0.0
    for r in res.results:
        zg = r["out_zg"].reshape(128, NKH, ITILES, 3).astype(np.float64)
        # row (it*128+p): z = sum_kh zg[p,kh,it,0]; g = both halves + sum_kh
        z = zg[:, :, :, 0].sum(axis=1)
        g = zg[:, :, :, 1:3].sum(axis=(1, 3))
        total += (g / z).sum(dtype=np.float64)
    total /= SC
    # term1 = sum_j u_p[j] . W1[j] -- the scalar part of the final reduction
    total += float(
        np.einsum("ij,ij->", u_p, ffn_w.reshape(N, 4, D)[:, 0, :], dtype=np.float64)
    )
    f = np.float32(max(total + float(np.asarray(ffn_b)[0]), 0.0))
    out = np.array([f], dtype=np.float32)
    if run_kwargs:
        return out, res
    return out


# revision 18
# speedup vs baseline: 1.0842x; 1.0842x over previous
"""Trainium2 Bass kernel for nn_CrossAttentionLayer (m=n=1024, d=2048).

Math:  f = relu(term1 + term23 + term4 + ffn_b), where with
W1..W4 = ffn_w.reshape(n, 4, d) per-candidate blocks:
  term1  = sum_i u_p[i] . W1[i]                      (host: tiny scalar dot)
  term23 = <softmax_rows(S),    (W2 + u_p*W3) @ u_c.T>_F     (S = [m,n] logits)
  term4  = <softmax_rows(S.T),  (u_c*W4)      @ u_p.T>_F
Row-constant offsets cancel inside row-softmax, and the remaining column
offset folds into the matmul:  softmax_k(S[i,:]) = softmax_k((u_p[i]*w3 + w2) @ u_c.T).

Both inner products have the identical SPMD shape
  result = sum_i [ sum_k exp(T[i,k]) * M2[i,k] ] / [ sum_k exp(T[i,k]) ]
  T  = ASp @ B.T,   M2 = C @ B.T
with ASp = A*w3 + a2 and C = C1 + A*C2 folded on the host, so 8 cores run
ONE program on different operands:
  cores 0-3 (mention shard I of 256): A=u_p[I], B=u_c, C=(W2+u_p*W3)[I], a2=w2
  cores 4-7 (candidate shard J):      A=u_c[J], B=u_p, C=(u_c*W4)[J],    a2=w1
Operands are pre-transposed ([d, rows]), quantized to fp8e4m3 with static
scales (SA for ASp, SC for C; exp() descales T via the activation's scale
input, the host descales gz by SC), and packed p-major [128, chunks*cols]
so every DMA is one contiguous multi-KB run per partition.  Matmuls run in
fp8 DoubleRow perf mode (2 contraction rows per partition per cycle).
Per-core outputs are tiny gz row-vectors; the host reduces them, adds
term1 + bias, and applies relu.
"""

import sys

sys.path.insert(0, "/opt/trn_rl_repo")

import ml_dtypes
import numpy as np

import concourse.bass as bass
import concourse.tile as tile
from concourse import mybir
from concourse.bass_utils import run_bass_kernel_spmd

F32 = mybir.dt.float32
F8 = mybir.dt.float8e4
NP_F8 = ml_dtypes.float8_e4m3

M = 1024  # mentions
N = 1024  # candidates
D = 2048  # feature dim (contraction)
NCORES = 8
ISH = 256  # per-core shard rows (A rows)
CH = D // 128  # 16 contraction chunks of 128
ITILES = ISH // 128  # 2
KH = 512  # rhs free-dim per matmul (PSUM bank width in fp32)
NKH = N // KH  # 2

SA = 32.0  # ASp fp8 scale (entries ~0.03 sigma -> ~1)
SC = 512.0  # C fp8 scale (entries ~0.0015 sigma -> ~0.8)

# ---------------------------------------------------------------------------
# Workaround: the pinned neuronxcc walrus accepts fewer sync waits per
# instruction than Tile's semaphore assignment attaches.  After scheduling,
# hoist excess waits of any over-capacity instruction onto same-engine
# EventSemaphores inserted right before it; each engine executes its stream
# in order, so the waits still gate the instruction.
_DEFAULT_CAP = 1
_WAIT_CAPS = {
    "InstTensorScalarPtr": 1,
    "InstTensorScalar": 1,
    "InstScalarTensorTensor": 1,
    "InstTensorReduce": 1,
}
_wfix_counter = [0]


def _legalize_waits(nc: bass.Bass) -> None:
    for f in nc.m.functions:
        for bb in f.blocks:
            il = bb.instructions
            out = []
            for inst in il:
                si = inst.sync_info
                waits = list(si.on_wait) if si and si.on_wait else []
                cap = _WAIT_CAPS.get(type(inst).__name__, _DEFAULT_CAP)
                if len(waits) > cap:
                    keep = waits[:cap]
                    for w in waits[cap:]:
                        _wfix_counter[0] += 1
                        out.append(
                            mybir.InstEventSemaphore(
                                name=f"I-wfix-{_wfix_counter[0]}",
                                engine=inst.engine,
                                ins=[],
                                outs=[],
                                sync_info=mybir.SyncInfo(on_wait=[w], on_update=[]),
                            )
                        )
                    inst.sync_info = mybir.SyncInfo(
                        on_wait=keep, on_update=list(si.on_update or [])
                    )
                out.append(inst)
            bb.instructions = out


# ---------------------------------------------------------------------------
def _emit(nc: bass.Bass, tc: tile.TileContext, io: dict) -> None:
    mult = mybir.AluOpType.mult
    add = mybir.AluOpType.add
    DR = mybir.MatmulPerfMode.DoubleRow

    a_r = io["a8"].ap().rearrange("p (c i) -> p c i", c=CH)
    c_r = io["c8"].ap().rearrange("p (c i) -> p c i", c=CH)
    b_r = io["b8"].ap().rearrange("p (h c k) -> p h c k", h=NKH, c=CH)
    zg_r = io["out_zg"].ap().rearrange("p (c z) -> p c z", z=3)

    import contextlib

    ctx = contextlib.ExitStack()
    singles = ctx.enter_context(tc.tile_pool(name="singles", bufs=1))
    scratch = ctx.enter_context(tc.tile_pool(name="scratch", bufs=3))
    psum = ctx.enter_context(tc.tile_pool(name="psum", bufs=2, space="PSUM"))

    a_sb = singles.tile([128, CH, ISH], F8)
    c_sb = singles.tile([128, CH, ISH], F8)
    b_sb = singles.tile([128, NKH, CH, KH], F8)
    # zg[:, col, 0] = Z partials, zg[:, col, 1:3] = G half partials (DVE and
    # GpSimd each reduce half the columns); host divides.
    zg = singles.tile([128, ITILES * NKH, 3], F32)

    # Tensor-engine warmup: the PE array boots in a half-speed p-state and
    # reaches full clock only after ~5 us of GAPLESS execution (idle gaps
    # reset the ramp).  Burn the DMA-wait window on one junk accumulation
    # chain -- accumulating matmuls pipeline back-to-back, so the ramp
    # carries straight into the real stream that follows on the engine.
    wa_sb = singles.tile([128, 2, 128], F8)
    wb_sb = singles.tile([128, 2, 256], F8)
    nc.vector.memset(wa_sb, 0.0)
    nc.vector.memset(wb_sb, 0.0)
    NWARM = 22
    wps = psum.tile([128, 256], F32, tag="tps0")
    for w in range(NWARM):
        nc.tensor.matmul(
            wps, lhsT=wa_sb, rhs=wb_sb, start=(w == 0), stop=(w == NWARM - 1),
            perf_mode=DR,
        )

    # Input DMAs: one queue per source tensor, pieces in consumption order
    # so supply stays ahead of the matmul stream.  Completion notify lags
    # data by ~2.5 us, so the pieces are small (4 chunks = 0.125 MB).
    QC = 4
    for q in range(CH // QC):
        sl = slice(q * QC, (q + 1) * QC)
        nc.sync.dma_start(out=a_sb[:, sl, :], in_=a_r[:, sl, :])
        nc.scalar.dma_start(out=c_sb[:, sl, :], in_=c_r[:, sl, :])
        nc.gpsimd.dma_start(out=b_sb[:, 0, sl, :], in_=b_r[:, 0, sl, :])
    for q in range(2):
        sl = slice(q * CH // 2, (q + 1) * CH // 2)
        nc.gpsimd.dma_start(out=b_sb[:, 1, sl, :], in_=b_r[:, 1, sl, :])

    # Main contraction: T and M2 accumulate over 8 DoubleRow chunks in PSUM,
    # then exp+rowsum (ACT, descaling by 1/SA) and mul+rowsum (DVE) fold k
    # away.  Each group's Z/G partial columns stream out as they finish.
    # Both it-tiles sweep each chunk together (4 matmuls per chunk pair) so
    # per-byte demand stays under the ~420 GB/s DMA supply -- a stall in the
    # tensor stream would also reset the p-state ramp.
    for kh in range(NKH):
        tps0 = psum.tile([128, KH], F32, tag="tps0")
        tps1 = psum.tile([128, KH], F32, tag="tps1")
        mps0 = psum.tile([128, KH], F32, tag="mps0")
        mps1 = psum.tile([128, KH], F32, tag="mps1")
        tps = [tps0, tps1]
        mps = [mps0, mps1]
        for c2 in range(CH // 2):
            sl2 = slice(2 * c2, 2 * c2 + 2)
            rhs = b_sb[:, kh, sl2, :]
            for it in range(ITILES):
                isl = slice(it * 128, (it + 1) * 128)
                nc.tensor.matmul(
                    tps[it],
                    lhsT=a_sb[:, sl2, isl],
                    rhs=rhs,
                    start=(c2 == 0),
                    stop=(c2 == CH // 2 - 1),
                    perf_mode=DR,
                )
                nc.tensor.matmul(
                    mps[it],
                    lhsT=c_sb[:, sl2, isl],
                    rhs=rhs,
                    start=(c2 == 0),
                    stop=(c2 == CH // 2 - 1),
                    perf_mode=DR,
                )
        for it in range(ITILES):
            col = kh * ITILES + it
            ep = scratch.tile([128, KH], F32, tag="ep")
            nc.scalar.activation(
                out=ep,
                in_=tps[it],
                func=mybir.ActivationFunctionType.Exp,
                scale=1.0 / SA,
                accum_out=zg[:, col, 0:1],
            )
            h2 = scratch.tile([128, KH], F32, tag="h")
            nc.vector.scalar_tensor_tensor(
                out=h2,
                in0=ep,
                scalar=1.0,
                in1=mps[it],
                op0=mult,
                op1=mult,
                accum_out=zg[:, col, 1:2],
            )
            nc.sync.dma_start(out=zg_r[:, col, :], in_=zg[:, col, :])
    ctx.close()


def _build() -> bass.Bass:
    nc = bass.Bass()
    io = {}
    io["a8"] = nc.declare_dram_parameter("a8", [128, CH * ISH], F8, isOutput=False)
    io["c8"] = nc.declare_dram_parameter("c8", [128, CH * ISH], F8, isOutput=False)
    io["b8"] = nc.declare_dram_parameter("b8", [128, NKH * CH * KH], F8, isOutput=False)
    io["out_zg"] = nc.declare_dram_parameter(
        "out_zg", [128, 3 * ITILES * NKH], F32, isOutput=True
    )
    with tile.TileContext(nc) as tc:
        _emit(nc, tc, io)
    _legalize_waits(nc)
    return nc


_NC_CACHE: bass.Bass | None = None


def _get_nc() -> bass.Bass:
    global _NC_CACHE
    if _NC_CACHE is None:
        _NC_CACHE = _build()
    return _NC_CACHE


def _q8(a2d: np.ndarray, scale: float) -> np.ndarray:
    return np.clip(a2d * scale, -240.0, 240.0).astype(NP_F8)


def _pack(a2d: np.ndarray) -> np.ndarray:
    """[D, x] (d-major) -> [128, CH*x] p-major (chunk-major per partition)."""
    x = a2d.shape[1]
    return np.ascontiguousarray(
        a2d.reshape(CH, 128, x).transpose(1, 0, 2).reshape(128, CH * x)
    )


def _pack_b(b2d: np.ndarray) -> np.ndarray:
    """[D, N] -> [128, NKH*CH*KH] with per-partition layout [kh][chunk][col]."""
    return np.ascontiguousarray(
        b2d.reshape(CH, 128, NKH, KH).transpose(1, 2, 0, 3).reshape(128, NKH * CH * KH)
    )


def _in_maps(u_p, u_c, w_a, ffn_w):
    u_pT = np.ascontiguousarray(u_p.T)
    u_cT = np.ascontiguousarray(u_c.T)
    W = ffn_w.reshape(N, 4, D)
    wa = w_a[0]
    w1, w2, w3 = wa[:D], wa[D : 2 * D], wa[2 * D :]

    # host-folded operands, [d, rows]
    asp_a = u_pT * w3[:, None] + w2[:, None]
    asp_b = u_cT * w3[:, None] + w1[:, None]
    c_a = W[:, 1, :].T + u_pT * W[:, 2, :].T  # W2 + u_p*W3
    c_b = u_cT * W[:, 3, :].T  # u_c*W4

    b8_a = _pack_b(_q8(u_cT, 1.0))
    b8_b = _pack_b(_q8(u_pT, 1.0))

    maps = []
    for grp, (asp, cc, b8) in enumerate(((asp_a, c_a, b8_a), (asp_b, c_b, b8_b))):
        for ci in range(4):
            sl = slice(ISH * ci, ISH * (ci + 1))
            maps.append(
                {
                    "a8": _pack(_q8(asp[:, sl], SA)),
                    "c8": _pack(_q8(cc[:, sl], SC)),
                    "b8": b8,
                }
            )
    return maps


def kernel(u_p, u_c, w_a, ffn_w, ffn_b, **run_kwargs):
    nc = _get_nc()
    u_p = np.asarray(u_p, np.float32)
    u_c = np.asarray(u_c, np.float32)
    w_a = np.asarray(w_a, np.float32)
    ffn_w = np.asarray(ffn_w, np.float32)
    maps = _in_maps(u_p, u_c, w_a, ffn_w)
    res = run_bass_kernel_spmd(nc, maps, core_ids=list(range(NCORES)), **run_kwargs)
    total = 0.0
    for r in res.results:
        zg = r["out_zg"].reshape(128, NKH, ITILES, 3).astype(np.float64)
        # row (it*128+p): z = sum_kh zg[p,kh,it,0]; g = both halves + sum_kh
        z = zg[:, :, :, 0].sum(axis=1)
        g = zg[:, :, :, 1:3].sum(axis=(1, 3))
        total += (g / z).sum(dtype=np.float64)
    total /= SC
    # term1 = sum_j u_p[j] . W1[j] -- the scalar part of the final reduction
    total += float(
        np.einsum("ij,ij->", u_p, ffn_w.reshape(N, 4, D)[:, 0, :], dtype=np.float64)
    )
    f = np.float32(max(total + float(np.asarray(ffn_b)[0]), 0.0))
    out = np.array([f], dtype=np.float32)
    if run_kwargs:
        return out, res
    return out
